# revision 18
# baseline (speedup 1.0000x reference)
"""Bass/Trainium2 kernel for nn_Attention_42305427865835.

Computes, for d_hidden [B,N,D], encoder_outputs [B,Lin,E], W1 [E+N*D, D],
b1 [D], w2 [D]:
    dec_proj = d_flat @ W1[:N*D] + b1                    # [B, D]   (host)
    enc_proj = enc @ W1[N*D:]                            # [B, Lin, E->D]
    scores   = tanh(enc_proj + dec_proj[:,None,:]) @ w2  # [B, Lin]
    out      = softmax(scores, axis=-1)

Sharding: data-parallel over batch, 4 batches per core on 8 cores.

Device dataflow (per core, batches b=0..3):
  - enc arrives transposed+scaled as fp8e4 [128(e%128), 4(e//128), 2048(l)].
  - enc_proj: fp8 DoubleRow matmuls (2 e-k-tiles per instruction, 2x PE
    throughput).  Weight-stationary pair-major order amortizes LDWEIGHTS,
    which the PE reorder window then hides entirely.  Outputs land in
    2-bank PSUM units [128, 2, 512] from a 3-deep ring.
  - tanh: one ACT instruction per PSUM unit ([128,1024]), per-partition
    bias = (dec_proj+b1) column, scale folds away the fp8 scaling.  The
    ACT engine is the steady-state bottleneck (~34us busy), so everything
    else is arranged to keep it saturated from ~13us on.
  - scoring: DVE per-partition multiplies by w2 (bf16, 4x mode) + add
    tree per l-half -> s_partial [128, 1024] x2; 16 "eye ones" matmuls
    (one per (b,lc)) accumulate partition sums into a single kernel-wide
    score tile [16, 512] PSUM (row 4b+lc).
  - softmax: ONE Exp over [16,512] with accum_out, one block-diag ones
    matmul for the 4 batch totals, one reciprocal + scale, one out DMA.
  - batch 0's enc DMA is split into l-halves and its matmuls run
    half-major so compute starts ~3us earlier; zero-matmul warmup keeps
    the PE HAM clock gate warm through the initial DMA.

dec_proj (16M MACs vs 17.2G total) is computed on the host during input
packing, like the weight transposes.
"""

import numpy as np

B, LIN, E, D, N = 32, 2048, 512, 512, 2
NCORES = 8
BPC = B // NCORES      # batches per core
P = 128                # SBUF partitions
ETILES = E // P        # 4 contraction k-tiles
DTILES = D // P        # 4 output j-tiles
ND = N * D             # 1024
LCH = 4                # Lin chunks per batch
LCHW = LIN // LCH      # 512 (one PSUM bank of fp32)
NROW = BPC * LCH       # 16 score rows

USE_FP8 = True
SE = 2.0 ** 5          # enc scale   (fp8e4 max ~240, |enc| < 6)
SW = 2.0 ** 12         # W1_e scale  (|W1_e| < 0.026)
NWARM = 10

# f32 blob layout (per partition)
DECB_OFF, DECB_LEN = 0, DTILES * BPC           # [j, b]
W2C_OFF, W2C_LEN = DECB_OFF + DECB_LEN, DTILES
BD_OFF, BD_LEN = W2C_OFF + W2C_LEN, NROW       # block-diag ones
WF32 = BD_OFF + BD_LEN

TRACE = False
TRACE_KWARGS = {}
LAST_RESULT = None

_CACHE = {}


def _build():
    import concourse.bacc as bacc
    import concourse.mybir as mybir
    import concourse.tile as tile

    f32 = mybir.dt.float32
    bf16 = mybir.dt.bfloat16
    fp8 = mybir.dt.float8e4
    AF = mybir.ActivationFunctionType
    DR = mybir.MatmulPerfMode.DoubleRow
    add_op = mybir.AluOpType.add

    enc_dt = fp8 if USE_FP8 else bf16
    tanh_scale = 1.0 / (SE * SW) if USE_FP8 else 1.0

    nc = bacc.Bacc("TRN2", target_bir_lowering=False)

    encC_h = nc.dram_tensor(
        "encC", [BPC, P, 2, ETILES, LIN // 2], enc_dt, kind="ExternalInput"
    )
    w1e_h = nc.dram_tensor("w1e", [P, ETILES, D], enc_dt, kind="ExternalInput")
    wf32_h = nc.dram_tensor("wf32", [P, WF32], f32, kind="ExternalInput")
    eyes_h = nc.dram_tensor("eyes", [P, NROW, NROW], bf16, kind="ExternalInput")
    out_h = nc.dram_tensor("out", [NROW, LCHW], f32, kind="ExternalOutput")

    with tile.TileContext(nc) as tc:
        with (
            tc.tile_pool(name="persist", bufs=1) as wp,
            tc.tile_pool(name="attnp", bufs=2) as attnp,
            tc.tile_pool(name="dvep", bufs=2) as dvep,
            tc.tile_pool(name="smp", bufs=1) as smp,
            tc.tile_pool(name="unitps", bufs=3, space="PSUM") as unitps,
            tc.tile_pool(name="scps", bufs=1, space="PSUM") as scps,
            tc.tile_pool(name="tps", bufs=1, space="PSUM") as tps,
        ):
            # --- DMA order tuned for the critical path: w1e and batch 0's
            # first l-half lead; eyes aren't needed until ~25us in ---
            HL = LIN // 2
            enc_sb = wp.tile([P, BPC, 2, ETILES, HL], enc_dt, tag="enc")
            nc.sync.dma_start(out=enc_sb[:, 0, 0], in_=encC_h[0][:, 0])

            w1e_sb = wp.tile([P, ETILES, D], enc_dt, tag="w1e")
            nc.sync.dma_start(out=w1e_sb, in_=w1e_h[:, :, :])

            wf32_sb = wp.tile([P, WF32], f32, tag="wf32")
            nc.sync.dma_start(out=wf32_sb, in_=wf32_h[:, :])

            nc.sync.dma_start(out=enc_sb[:, 0, 1], in_=encC_h[0][:, 1])
            nc.sync.dma_start(out=enc_sb[:, 1], in_=encC_h[1])

            eyes_sb = wp.tile([P, NROW, NROW], bf16, tag="eyes")
            nc.sync.dma_start(out=eyes_sb, in_=eyes_h[:, :, :])

            for b in range(2, BPC):
                nc.sync.dma_start(out=enc_sb[:, b], in_=encC_h[b])

            decb_sb = wf32_sb[:, DECB_OFF : DECB_OFF + DECB_LEN].rearrange(
                "p (j b) -> p j b", j=DTILES
            )
            w2c_sb = wf32_sb[:, W2C_OFF : W2C_OFF + W2C_LEN]
            bdiag_sb = wf32_sb[:, BD_OFF : BD_OFF + BD_LEN]

            zero_sb = wp.tile([P, LCHW], bf16, tag="zeros")
            nc.vector.memset(zero_sb, 0.0)
            sumexps = wp.tile([P, 1], f32, tag="sumexps")
            nc.vector.memset(sumexps, 0.0)

            # --- PE warm-up during the initial enc DMA (HAM clock gate) ---
            wps = tps.tile([P, LCHW], f32, tag="T", name="warm")
            for i in range(NWARM):
                nc.tensor.matmul(
                    out=wps,
                    lhsT=zero_sb[:, 0:P],
                    rhs=zero_sb,
                    start=(i == 0),
                    stop=(i == NWARM - 1),
                )

            attn = [None] * BPC     # [P, DTILES, 2, 2*LCHW] bf16 per batch
            tmul = [[[None] * DTILES for _ in range(2)] for _ in range(BPC)]
            sph = [[None, None] for _ in range(BPC)]   # summed partials per l-half
            score = scps.tile([NROW, LCHW], f32, tag="sc")
            NEYE = 12 + 2 + 2 * DTILES  # total score-accumulating matmuls
            eye_count = [0]

            def eye_mm(row, rhs):
                eye_count[0] += 1
                nc.tensor.matmul(
                    out=score,
                    lhsT=eyes_sb[:, row, :],
                    rhs=rhs,
                    start=(eye_count[0] == 1),
                    stop=(eye_count[0] == NEYE),
                )

            def emit_mms(b, j, halves):
                """enc_proj matmuls for (b, j) over the given l-halves."""
                chunks = [c for h in halves for c in (2 * h, 2 * h + 1)]
                units = {}
                for h in halves:
                    units[h] = unitps.tile(
                        [P, 2, LCHW], f32, tag="u", name=f"u{b}_{j}_{h}"
                    )
                if USE_FP8:
                    for pair in range(2):
                        lhsT = w1e_sb[:, 2 * pair : 2 * pair + 2, j * P : (j + 1) * P]
                        for c in chunks:
                            nc.tensor.matmul(
                                out=units[c // 2][:, c % 2, :],
                                lhsT=lhsT,
                                rhs=enc_sb[
                                    :, b, c // 2, 2 * pair : 2 * pair + 2,
                                    (c % 2) * LCHW : (c % 2 + 1) * LCHW,
                                ],
                                start=(pair == 0),
                                stop=(pair == 1),
                                perf_mode=DR,
                            )
                else:
                    for e in range(ETILES):
                        lhsT = w1e_sb[:, e, j * P : (j + 1) * P]
                        for c in chunks:
                            nc.tensor.matmul(
                                out=units[c // 2][:, c % 2, :],
                                lhsT=lhsT,
                                rhs=enc_sb[
                                    :, b, c // 2, e,
                                    (c % 2) * LCHW : (c % 2 + 1) * LCHW,
                                ],
                                start=(e == 0),
                                stop=(e == ETILES - 1),
                            )
                return units

            def emit_act(b, j, h, unit):
                """tanh + w2-multiply for one unit; add tree when ready."""
                if attn[b] is None:
                    attn[b] = attnp.tile(
                        [P, DTILES, 2, 2 * LCHW], bf16, tag="attn", name=f"attn{b}"
                    )
                nc.scalar.activation(
                    out=attn[b][:, j, h, :],
                    in_=unit.rearrange("p a b -> p (a b)"),
                    func=AF.Tanh,
                    bias=decb_sb[:, j, b : b + 1],
                    scale=tanh_scale,
                )
                t = dvep.tile(
                    [P, 2 * LCHW], bf16, tag="t", bufs=6, name=f"t{b}{j}{h}"
                )
                nc.vector.tensor_scalar_mul(
                    out=t, in0=attn[b][:, j, h, :], scalar1=w2c_sb[:, j : j + 1]
                )
                tmul[b][h][j] = t
                if b == BPC - 1 and h == 1:
                    return  # last batch's h1 scores go per-j via eye_mm
                tm = tmul[b][h]
                if tm[0] is not None and tm[1] is not None and tm[2] is None:
                    a01 = dvep.tile(
                        [P, 2 * LCHW], bf16, tag="aa", bufs=4, name=f"a01_{b}{h}"
                    )
                    nc.vector.tensor_tensor(out=a01, in0=tm[0], in1=tm[1], op=add_op)
                    tm[0] = a01
                if tm[2] is not None and tm[3] is not None:
                    a23 = dvep.tile(
                        [P, 2 * LCHW], bf16, tag="aa", bufs=4, name=f"a23_{b}{h}"
                    )
                    nc.vector.tensor_tensor(out=a23, in0=tm[2], in1=tm[3], op=add_op)
                    s = dvep.tile(
                        [P, 2 * LCHW], bf16, tag="sph", bufs=3, name=f"sp{b}{h}"
                    )
                    nc.vector.tensor_tensor(out=s, in0=tm[0], in1=a23, op=add_op)
                    sph[b][h] = s

            def emit_eyes(b, h):
                """Partition-reduce one l-half of batch b into score rows."""
                for lc in (2 * h, 2 * h + 1):
                    eye_mm(
                        b * LCH + lc,
                        sph[b][h][:, (lc - 2 * h) * LCHW : (lc - 2 * h + 1) * LCHW],
                    )

            def emit_eyes_perj(j):
                """Last batch, h1: accumulate w2-multiplied tiles directly."""
                bl = BPC - 1
                for lc in (2, 3):
                    eye_mm(
                        bl * LCH + lc,
                        tmul[bl][1][j][:, (lc - 2) * LCHW : (lc - 1) * LCHW],
                    )

            # --- main pipeline ---
            # batch 0 runs l-half-major so it only needs the first half-DMA;
            # eye matmuls for batch b ride inside batch b+1's PE stream.
            for b in range(BPC):
                if b == 0:
                    work = [(j, (0,)) for j in range(DTILES)] + [
                        (j, (1,)) for j in range(DTILES)
                    ]
                else:
                    work = [(j, (0, 1)) for j in range(DTILES)]
                for wi, (j, halves) in enumerate(work):
                    units = emit_mms(b, j, halves)
                    if b > 0:
                        if wi == 0:
                            emit_eyes(b - 1, 0)
                        elif wi == 1:
                            emit_eyes(b - 1, 1)
                    if b == BPC - 1 and wi > 0:
                        emit_eyes_perj(wi - 1)  # h1 scores of the previous j
                    for h in halves:
                        emit_act(b, j, h, units[h])
            emit_eyes(BPC - 1, 0)
            emit_eyes_perj(DTILES - 1)

            # --- softmax epilogue: one exp, one total-matmul, one scale ---
            erow = smp.tile([NROW, LCHW], f32, tag="erow")
            nc.scalar.activation(
                out=erow,
                in_=score,
                func=AF.Exp,
                bias=0.0,
                scale=1.0,
                accum_out=sumexps[0:NROW, :],
            )
            tot = scps.tile([NROW, 1], f32, tag="sc", name="tot")
            nc.tensor.matmul(
                out=tot, lhsT=bdiag_sb, rhs=sumexps[:, :], start=True, stop=True
            )
            rinv = smp.tile([NROW, 1], f32, tag="rinv")
            nc.vector.reciprocal(out=rinv, in_=tot)
            orow = smp.tile([NROW, LCHW], f32, tag="orow")
            nc.vector.tensor_scalar_mul(out=orow, in0=erow, scalar1=rinv)
            nc.sync.dma_start(out=out_h[:, :], in_=orow)

    nc.compile()
    return nc


def _prep_in_maps(d_hidden, encoder_outputs, W1, b1, w2):
    import ml_dtypes

    bf = ml_dtypes.bfloat16
    f8 = ml_dtypes.float8_e4m3
    enc_np = f8 if USE_FP8 else bf

    d_hidden = np.ascontiguousarray(np.asarray(d_hidden), dtype=np.float32)
    encoder_outputs = np.ascontiguousarray(
        np.asarray(encoder_outputs), dtype=np.float32
    )
    W1 = np.ascontiguousarray(np.asarray(W1), dtype=np.float32)
    b1 = np.ascontiguousarray(np.asarray(b1), dtype=np.float32)
    w2 = np.ascontiguousarray(np.asarray(w2), dtype=np.float32)

    W1d, W1e = W1[:ND], W1[ND:]
    dec = d_hidden.reshape(B, ND) @ W1d + b1    # [B, D] on host

    w1e_scaled = W1e * (SW if USE_FP8 else 1.0)
    w1e = np.ascontiguousarray(
        w1e_scaled.reshape(ETILES, P, D).transpose(1, 0, 2).astype(enc_np)
    )
    eyes = np.zeros((P, NROW, NROW), dtype=bf)
    for r in range(NROW):
        eyes[:, r, r] = 1.0

    enc_scale = SE if USE_FP8 else 1.0
    in_maps = []
    for c in range(NCORES):
        bs = slice(c * BPC, (c + 1) * BPC)
        # [BPC, P, 2, ETILES, LIN//2]: encC[b, p, hf, et, l] =
        #   enc[b, hf*LIN//2 + l, et*P+p] * SE  (contiguous per l-half)
        encC = np.ascontiguousarray(
            (encoder_outputs[bs] * enc_scale)
            .transpose(0, 2, 1)
            .reshape(BPC, ETILES, P, 2, LIN // 2)
            .transpose(0, 2, 3, 1, 4)
            .astype(enc_np)
        )
        wf32 = np.zeros((P, WF32), dtype=np.float32)
        # decb[p, j*BPC+b] = dec[b, j*P+p]
        wf32[:, DECB_OFF : DECB_OFF + DECB_LEN] = (
            dec[bs].reshape(BPC, DTILES, P).transpose(2, 1, 0).reshape(P, DECB_LEN)
        )
        wf32[:, W2C_OFF : W2C_OFF + W2C_LEN] = w2.reshape(DTILES, P).T
        for r in range(NROW):
            wf32[4 * (r // LCH) : 4 * (r // LCH) + 4, BD_OFF + r] = 1.0
        in_maps.append({"encC": encC, "w1e": w1e, "wf32": wf32, "eyes": eyes})
    return in_maps


def kernel(d_hidden, encoder_outputs, W1, b1, w2):
    global LAST_RESULT
    from concourse import bass_utils

    if "nc" not in _CACHE:
        _CACHE["nc"] = _build()
    nc = _CACHE["nc"]

    in_maps = _prep_in_maps(d_hidden, encoder_outputs, W1, b1, w2)
    res = bass_utils.run_bass_kernel_spmd(
        nc,
        in_maps,
        core_ids=list(range(NCORES)),
        trace=TRACE,
        **TRACE_KWARGS,
    )
    LAST_RESULT = res
    return np.concatenate(
        [r["out"].reshape(BPC, LIN) for r in res.results], axis=0
    ).astype(np.float32)


# revision 19
# speedup vs baseline: 1.2089x; 1.2089x over previous
"""Bass/Trainium2 kernel for nn_Attention_42305427865835.

Computes, for d_hidden [B,N,D], encoder_outputs [B,Lin,E], W1 [E+N*D, D],
b1 [D], w2 [D]:
    dec_proj = d_flat @ W1[:N*D] + b1                    # [B, D]   (host)
    enc_proj = enc @ W1[N*D:]                            # [B, Lin, E->D]
    scores   = tanh(enc_proj + dec_proj[:,None,:]) @ w2  # [B, Lin]
    out      = softmax(scores, axis=-1)

Sharding: data-parallel over batch, 4 batches per core on 8 cores.

Device dataflow (per core, batches b=0..3):
  - enc arrives transposed+scaled as fp8e4 [128(e%128), 4(e//128), 2048(l)].
  - enc_proj: fp8 DoubleRow matmuls (2 e-k-tiles per instruction, 2x PE
    throughput).  Weight-stationary pair-major order amortizes LDWEIGHTS,
    which the PE reorder window then hides entirely.  Outputs land in
    2-bank PSUM units [128, 2, 512] from a 3-deep ring.
  - tanh: one ACT instruction per PSUM unit ([128,1024]), per-partition
    bias = (dec_proj+b1) column, scale folds away the fp8 scaling.  The
    ACT engine is the steady-state bottleneck (~34us busy), so everything
    else is arranged to keep it saturated from ~13us on.
  - scoring: DVE per-partition multiplies by w2 (bf16, 4x mode) + add
    tree per l-half -> s_partial [128, 1024] x2; 16 "eye ones" matmuls
    (one per (b,lc)) accumulate partition sums into a single kernel-wide
    score tile [16, 512] PSUM (row 4b+lc).
  - softmax: ONE Exp over [16,512] with accum_out, one block-diag ones
    matmul for the 4 batch totals, one reciprocal + scale, one out DMA.
  - batch 0's enc DMA is split into l-halves and its matmuls run
    half-major so compute starts ~3us earlier; zero-matmul warmup keeps
    the PE HAM clock gate warm through the initial DMA.

dec_proj (16M MACs vs 17.2G total) is computed on the host during input
packing, like the weight transposes.
"""

import numpy as np

B, LIN, E, D, N = 32, 2048, 512, 512, 2
NCORES = 8
BPC = B // NCORES      # batches per core
P = 128                # SBUF partitions
ETILES = E // P        # 4 contraction k-tiles
DTILES = D // P        # 4 output j-tiles
ND = N * D             # 1024
LCH = 4                # Lin chunks per batch
LCHW = LIN // LCH      # 512 (one PSUM bank of fp32)
NROW = BPC * LCH       # 16 score rows

USE_FP8 = True
SE = 2.0 ** 5          # enc scale   (fp8e4 max ~240, |enc| < 6)
SW = 2.0 ** 12         # W1_e scale  (|W1_e| < 0.026)
NWARM = 10

# f32 blob layout (per partition)
DECB_OFF, DECB_LEN = 0, DTILES * BPC           # [j, b]
W2C_OFF, W2C_LEN = DECB_OFF + DECB_LEN, DTILES
BD_OFF, BD_LEN = W2C_OFF + W2C_LEN, NROW       # block-diag ones
WF32 = BD_OFF + BD_LEN

TRACE = False
TRACE_KWARGS = {}
LAST_RESULT = None

_CACHE = {}


def _build():
    import concourse.bacc as bacc
    import concourse.mybir as mybir
    import concourse.tile as tile

    f32 = mybir.dt.float32
    bf16 = mybir.dt.bfloat16
    fp8 = mybir.dt.float8e4
    AF = mybir.ActivationFunctionType
    DR = mybir.MatmulPerfMode.DoubleRow
    add_op = mybir.AluOpType.add

    enc_dt = fp8 if USE_FP8 else bf16
    tanh_scale = 1.0 / (SE * SW) if USE_FP8 else 1.0

    nc = bacc.Bacc("TRN2", target_bir_lowering=False)

    encC_h = nc.dram_tensor("encC", [BPC, P, ETILES, LIN], enc_dt, kind="ExternalInput")
    w1e_h = nc.dram_tensor("w1e", [P, ETILES, D], enc_dt, kind="ExternalInput")
    wf32_h = nc.dram_tensor("wf32", [P, WF32], f32, kind="ExternalInput")
    eyes_h = nc.dram_tensor("eyes", [P, NROW, NROW], bf16, kind="ExternalInput")
    out_h = nc.dram_tensor("out", [NROW, LCHW], f32, kind="ExternalOutput")

    with tile.TileContext(nc) as tc:
        with (
            tc.tile_pool(name="persist", bufs=1) as wp,
            tc.tile_pool(name="attnp", bufs=2) as attnp,
            tc.tile_pool(name="dvep", bufs=2) as dvep,
            tc.tile_pool(name="smp", bufs=1) as smp,
            tc.tile_pool(name="unitps", bufs=3, space="PSUM") as unitps,
            tc.tile_pool(name="scps", bufs=1, space="PSUM") as scps,
            tc.tile_pool(name="tps", bufs=1, space="PSUM") as tps,
        ):
            # --- DMA order tuned for the critical path: w1e and batch 0's
            # first l-half lead; eyes aren't needed until ~25us in ---
            HL = LIN // 2
            enc_sb = wp.tile([P, BPC, ETILES, LIN], enc_dt, tag="enc")
            nc.sync.dma_start(out=enc_sb[:, 0, :, 0:HL], in_=encC_h[0][:, :, 0:HL])

            w1e_sb = wp.tile([P, ETILES, D], enc_dt, tag="w1e")
            nc.sync.dma_start(out=w1e_sb, in_=w1e_h[:, :, :])

            wf32_sb = wp.tile([P, WF32], f32, tag="wf32")
            nc.sync.dma_start(out=wf32_sb, in_=wf32_h[:, :])

            nc.sync.dma_start(out=enc_sb[:, 0, :, HL:LIN], in_=encC_h[0][:, :, HL:LIN])
            nc.sync.dma_start(out=enc_sb[:, 1], in_=encC_h[1])

            eyes_sb = wp.tile([P, NROW, NROW], bf16, tag="eyes")
            nc.sync.dma_start(out=eyes_sb, in_=eyes_h[:, :, :])

            for b in range(2, BPC):
                nc.sync.dma_start(out=enc_sb[:, b], in_=encC_h[b])

            decb_sb = wf32_sb[:, DECB_OFF : DECB_OFF + DECB_LEN].rearrange(
                "p (j b) -> p j b", j=DTILES
            )
            w2c_sb = wf32_sb[:, W2C_OFF : W2C_OFF + W2C_LEN]
            bdiag_sb = wf32_sb[:, BD_OFF : BD_OFF + BD_LEN]

            zero_sb = wp.tile([P, LCHW], bf16, tag="zeros")
            nc.vector.memset(zero_sb, 0.0)
            sumexps = wp.tile([P, 1], f32, tag="sumexps")
            nc.vector.memset(sumexps, 0.0)

            # --- PE warm-up during the initial enc DMA (HAM clock gate) ---
            wps = tps.tile([P, LCHW], f32, tag="T", name="warm")
            for i in range(NWARM):
                nc.tensor.matmul(
                    out=wps,
                    lhsT=zero_sb[:, 0:P],
                    rhs=zero_sb,
                    start=(i == 0),
                    stop=(i == NWARM - 1),
                )

            attn = [None] * BPC     # [P, DTILES, 2, 2*LCHW] bf16 per batch
            tmul = [[[None] * DTILES for _ in range(2)] for _ in range(BPC)]
            sph = [[None, None] for _ in range(BPC)]   # summed partials per l-half
            score = scps.tile([NROW, LCHW], f32, tag="sc")
            NEYE = 12 + 2 + 2 * DTILES  # total score-accumulating matmuls
            eye_count = [0]

            def eye_mm(row, rhs):
                eye_count[0] += 1
                nc.tensor.matmul(
                    out=score,
                    lhsT=eyes_sb[:, row, :],
                    rhs=rhs,
                    start=(eye_count[0] == 1),
                    stop=(eye_count[0] == NEYE),
                )

            def emit_mms(b, j, halves):
                """enc_proj matmuls for (b, j) over the given l-halves."""
                chunks = [c for h in halves for c in (2 * h, 2 * h + 1)]
                units = {}
                for h in halves:
                    units[h] = unitps.tile(
                        [P, 2, LCHW], f32, tag="u", name=f"u{b}_{j}_{h}"
                    )
                if USE_FP8:
                    for pair in range(2):
                        lhsT = w1e_sb[:, 2 * pair : 2 * pair + 2, j * P : (j + 1) * P]
                        for c in chunks:
                            nc.tensor.matmul(
                                out=units[c // 2][:, c % 2, :],
                                lhsT=lhsT,
                                rhs=enc_sb[
                                    :, b, 2 * pair : 2 * pair + 2,
                                    c * LCHW : (c + 1) * LCHW,
                                ],
                                start=(pair == 0),
                                stop=(pair == 1),
                                perf_mode=DR,
                            )
                else:
                    for e in range(ETILES):
                        lhsT = w1e_sb[:, e, j * P : (j + 1) * P]
                        for c in chunks:
                            nc.tensor.matmul(
                                out=units[c // 2][:, c % 2, :],
                                lhsT=lhsT,
                                rhs=enc_sb[:, b, e, c * LCHW : (c + 1) * LCHW],
                                start=(e == 0),
                                stop=(e == ETILES - 1),
                            )
                return units

            def emit_act(b, j, h, unit):
                """tanh + w2-multiply for one unit; add tree when ready."""
                if attn[b] is None:
                    attn[b] = attnp.tile(
                        [P, DTILES, 2, 2 * LCHW], bf16, tag="attn", name=f"attn{b}"
                    )
                nc.scalar.activation(
                    out=attn[b][:, j, h, :],
                    in_=unit.rearrange("p a b -> p (a b)"),
                    func=AF.Tanh,
                    bias=decb_sb[:, j, b : b + 1],
                    scale=tanh_scale,
                )
                t = dvep.tile(
                    [P, 2 * LCHW], bf16, tag="t", bufs=6, name=f"t{b}{j}{h}"
                )
                nc.vector.tensor_scalar_mul(
                    out=t, in0=attn[b][:, j, h, :], scalar1=w2c_sb[:, j : j + 1]
                )
                tmul[b][h][j] = t
                if b == BPC - 1 and h == 1:
                    return  # last batch's h1 scores go per-j via eye_mm
                tm = tmul[b][h]
                if tm[0] is not None and tm[1] is not None and tm[2] is None:
                    a01 = dvep.tile(
                        [P, 2 * LCHW], bf16, tag="aa", bufs=4, name=f"a01_{b}{h}"
                    )
                    nc.vector.tensor_tensor(out=a01, in0=tm[0], in1=tm[1], op=add_op)
                    tm[0] = a01
                if tm[2] is not None and tm[3] is not None:
                    a23 = dvep.tile(
                        [P, 2 * LCHW], bf16, tag="aa", bufs=4, name=f"a23_{b}{h}"
                    )
                    nc.vector.tensor_tensor(out=a23, in0=tm[2], in1=tm[3], op=add_op)
                    s = dvep.tile(
                        [P, 2 * LCHW], bf16, tag="sph", bufs=3, name=f"sp{b}{h}"
                    )
                    nc.vector.tensor_tensor(out=s, in0=tm[0], in1=a23, op=add_op)
                    sph[b][h] = s

            def emit_eyes(b, h):
                """Partition-reduce one l-half of batch b into score rows."""
                for lc in (2 * h, 2 * h + 1):
                    eye_mm(
                        b * LCH + lc,
                        sph[b][h][:, (lc - 2 * h) * LCHW : (lc - 2 * h + 1) * LCHW],
                    )

            def emit_eyes_perj(j):
                """Last batch, h1: accumulate w2-multiplied tiles directly."""
                bl = BPC - 1
                for lc in (2, 3):
                    eye_mm(
                        bl * LCH + lc,
                        tmul[bl][1][j][:, (lc - 2) * LCHW : (lc - 1) * LCHW],
                    )

            # --- main pipeline ---
            # batch 0 runs l-half-major so it only needs the first half-DMA;
            # eye matmuls for batch b ride inside batch b+1's PE stream.
            for b in range(BPC):
                if b == 0:
                    work = [(j, (0,)) for j in range(DTILES)] + [
                        (j, (1,)) for j in range(DTILES)
                    ]
                else:
                    work = [(j, (0, 1)) for j in range(DTILES)]
                for wi, (j, halves) in enumerate(work):
                    units = emit_mms(b, j, halves)
                    if b > 0:
                        if wi == 0:
                            emit_eyes(b - 1, 0)
                        elif wi == 1:
                            emit_eyes(b - 1, 1)
                    if b == BPC - 1 and wi > 0:
                        emit_eyes_perj(wi - 1)  # h1 scores of the previous j
                    for h in halves:
                        emit_act(b, j, h, units[h])
            emit_eyes(BPC - 1, 0)
            emit_eyes_perj(DTILES - 1)

            # --- softmax epilogue: one exp, one total-matmul, one scale ---
            erow = smp.tile([NROW, LCHW], f32, tag="erow")
            nc.scalar.activation(
                out=erow,
                in_=score,
                func=AF.Exp,
                bias=0.0,
                scale=1.0,
                accum_out=sumexps[0:NROW, :],
            )
            tot = scps.tile([NROW, 1], f32, tag="sc", name="tot")
            nc.tensor.matmul(
                out=tot, lhsT=bdiag_sb, rhs=sumexps[:, :], start=True, stop=True
            )
            rinv = smp.tile([NROW, 1], f32, tag="rinv")
            nc.vector.reciprocal(out=rinv, in_=tot)
            orow = smp.tile([NROW, LCHW], f32, tag="orow")
            nc.vector.tensor_scalar_mul(out=orow, in0=erow, scalar1=rinv)
            nc.sync.dma_start(out=out_h[:, :], in_=orow)

    nc.compile()
    return nc


def _prep_in_maps(d_hidden, encoder_outputs, W1, b1, w2):
    import ml_dtypes

    bf = ml_dtypes.bfloat16
    f8 = ml_dtypes.float8_e4m3
    enc_np = f8 if USE_FP8 else bf

    d_hidden = np.ascontiguousarray(np.asarray(d_hidden), dtype=np.float32)
    encoder_outputs = np.ascontiguousarray(
        np.asarray(encoder_outputs), dtype=np.float32
    )
    W1 = np.ascontiguousarray(np.asarray(W1), dtype=np.float32)
    b1 = np.ascontiguousarray(np.asarray(b1), dtype=np.float32)
    w2 = np.ascontiguousarray(np.asarray(w2), dtype=np.float32)

    W1d, W1e = W1[:ND], W1[ND:]
    dec = d_hidden.reshape(B, ND) @ W1d + b1    # [B, D] on host

    w1e_scaled = W1e * (SW if USE_FP8 else 1.0)
    w1e = np.ascontiguousarray(
        w1e_scaled.reshape(ETILES, P, D).transpose(1, 0, 2).astype(enc_np)
    )
    eyes = np.zeros((P, NROW, NROW), dtype=bf)
    for r in range(NROW):
        eyes[:, r, r] = 1.0

    enc_scale = SE if USE_FP8 else 1.0
    in_maps = []
    for c in range(NCORES):
        bs = slice(c * BPC, (c + 1) * BPC)
        # [BPC, P, ETILES, LIN]: encC[b, p, et, l] = enc[b, l, et*P+p] * SE
        encC = np.ascontiguousarray(
            (encoder_outputs[bs] * enc_scale)
            .transpose(0, 2, 1)
            .reshape(BPC, ETILES, P, LIN)
            .transpose(0, 2, 1, 3)
            .astype(enc_np)
        )
        wf32 = np.zeros((P, WF32), dtype=np.float32)
        # decb[p, j*BPC+b] = dec[b, j*P+p]
        wf32[:, DECB_OFF : DECB_OFF + DECB_LEN] = (
            dec[bs].reshape(BPC, DTILES, P).transpose(2, 1, 0).reshape(P, DECB_LEN)
        )
        wf32[:, W2C_OFF : W2C_OFF + W2C_LEN] = w2.reshape(DTILES, P).T
        for r in range(NROW):
            wf32[4 * (r // LCH) : 4 * (r // LCH) + 4, BD_OFF + r] = 1.0
        in_maps.append({"encC": encC, "w1e": w1e, "wf32": wf32, "eyes": eyes})
    return in_maps


def kernel(d_hidden, encoder_outputs, W1, b1, w2):
    global LAST_RESULT
    from concourse import bass_utils

    if "nc" not in _CACHE:
        _CACHE["nc"] = _build()
    nc = _CACHE["nc"]

    in_maps = _prep_in_maps(d_hidden, encoder_outputs, W1, b1, w2)
    res = bass_utils.run_bass_kernel_spmd(
        nc,
        in_maps,
        core_ids=list(range(NCORES)),
        trace=TRACE,
        **TRACE_KWARGS,
    )
    LAST_RESULT = res
    return np.concatenate(
        [r["out"].reshape(BPC, LIN) for r in res.results], axis=0
    ).astype(np.float32)
